# revision 72
# baseline (speedup 1.0000x reference)
"""Trainium2 Bass kernel for nn_Attention_86199993631321.

Reference computation (B=8, N=128, H=512):
    pair[b,i,j,:] = x[b,i,:] + x[b,j,:]
    out = pair @ W.T + b                # [B, N, N, H]

Key algebraic simplification: the Linear is applied to a *sum*, so
    out[b,i,j,:] = P[b,i,:] + P[b,j,:]   where P = x @ W.T + 0.5*b
This turns 68.7 GFLOP of einsum into a 0.5 GFLOP matmul plus a broadcast-add
that only has to *write* the 268 MB output.

Sharding: pure data-parallel over batch B (core b handles batch b), no
collectives.  Per core:
  - P = x_b @ W.T + 0.5*b via TensorE (inputs pre-transposed and packed on
    host; the bias folds in as a K=1 matmul of a ones-row with 0.5*b).
  - P's rows are staged (bf16) into PE row-group base partitions {0,32,64,96};
    K=1 rank-1 matmuls ones ⊗ P[j,:] broadcast each row across partitions
    into PSUM, 4 row groups running concurrently in the array.
  - PSUM tiles are evicted as bf16 (ScalarE activation-copy or VectorE copy)
    and the i-term is added by bf16 2x-mode tensor_tensor on DVE or GpSimd.
  - out is symmetric (out[i,j]=out[j,i]), which saves 25% of the vector-engine
    work: j<64 tiles are computed at full height and ALSO written to their
    mirrored location (rows j, cols i>=64); the (i>=64, j>=64) quadrant is
    computed as half-height tiles packed two-per-PSUM-tile.
  - out lands in HBM as bf16 (half the write traffic); the host upcasts.
"""

import sys

if "/opt/trn_rl_repo" not in sys.path:
    sys.path.insert(0, "/opt/trn_rl_repo")

import numpy as np

B, N, H = 8, 128, 512
NCORES = 8
KC = H // 128  # contraction chunks for the P matmul
JBLK = 8       # j rows per output tile
TTW = 4        # j rows per PSUM tile ([128, TTW*H] = 4 banks)
NQ = 4         # PE row-group quadrants
RPQ = JBLK // NQ  # rows per quadrant in a chunk (2)
HN = N // 2    # 64
# packed input layout (per core, bf16): wx[h, 0:128] = x.T,
# wx[h, 128:640] = W.T, wx[0, 640:768] = 1.0 (ones row for the bias matmul)
WXW = N + H + 128
# Per-group eviction route (24 groups): SV = ACT copy + DVE TT, VV = DVE
# copy + DVE TT, PS/PV = the PE adds P itself via an identity matmul
# accumulation and the result needs only a copy (ACT / DVE).  The PE is
# mostly idle, so P-routes convert vector-engine work into array time.
# NOTE: GpSimd tensor_tensor (SG) is banned here — a concurrent GpSimd
# 2-input op steals DVE SBUF ports and degrades simultaneous DVE TTs from
# ~1.2us to ~3-5us (measured), so SG routes cost more than they save.
ROUTES = [
    "SV", "SV", "SV", "SV", "SV", "SV", "SV", "VV",
    "SV", "SV", "SV", "SV", "SV", "SV", "SV", "VV",
    "SV", "SV", "SV", "SV", "SV", "SV", "SV", "SV",
]

_BUILT = {}


def _build_nc():
    import concourse.bass as bass
    import concourse.bacc as bacc
    import concourse.tile as tile
    from concourse import mybir
    from concourse.masks import make_identity

    f32 = mybir.dt.float32
    bf16 = mybir.dt.bfloat16

    nc = bacc.Bacc()
    wx_ext = nc.declare_dram_parameter("wx", [H, WXW], bf16, isOutput=False)
    hb_ext = nc.declare_dram_parameter("halfb", [1, H], bf16, isOutput=False)
    out_ext = nc.declare_dram_parameter("out", [N, N, H], bf16, isOutput=True)

    group_idx = [0]

    with tile.TileContext(nc) as tc:
        with (
            tc.tile_pool(name="const", bufs=1) as const,
            tc.tile_pool(name="stage", bufs=6) as stage,
            tc.tile_pool(name="bcast", bufs=8) as bcast,
            tc.tile_pool(name="outp", bufs=6) as outp,
            tc.tile_pool(name="psum", bufs=2, space="PSUM") as psum,
        ):
            # ---- load packed inputs ----
            wx_sb = const.tile([128, KC, WXW], bf16)  # [h_local, (kc, m)]
            wx_v = wx_ext.rearrange("(c p) m -> p c m", p=128)
            # per-chunk DMAs on both HWDGE rings: the first proj matmul can
            # start as soon as its own K-chunk has landed
            for c in range(KC):
                eng = nc.sync if c % 2 == 0 else nc.scalar
                eng.dma_start(out=wx_sb[:, c, :], in_=wx_v[:, c, :])
            ones_sb = const.tile([128, 128], bf16)
            nc.vector.memset(ones_sb, 1.0)
            hb_sb = const.tile([1, H], bf16)
            nc.sync.dma_start(out=hb_sb, in_=hb_ext[:, :])

            # ---- P = x @ W.T + 0.5*b -> PSUM [128(i), 512(o)] ----
            ps_proj = psum.tile([128, TTW * H], f32, tag="ps")
            for c in range(KC):
                nc.tensor.matmul(
                    ps_proj[:, 0:H],
                    wx_sb[:, c, 0:N],
                    wx_sb[:, c, N : N + H],
                    start=(c == 0),
                    stop=False,
                )
            nc.tensor.matmul(
                ps_proj[:, 0:H],
                wx_sb[0:1, 0, N + H : N + H + 128],
                hb_sb,
                start=False,
                stop=True,
            )

            # P replicated 4x along the free dim (bf16): TT in0 + staging src
            P_rep = const.tile([128, TTW, H], bf16)
            for u in range(TTW):
                nc.scalar.activation(
                    P_rep[:, u, :],
                    ps_proj[:, 0:H],
                    mybir.ActivationFunctionType.Copy,
                )
            # stacked upper-half copy for the packed (i>=64) quadrant tiles:
            # P_stk[p] = P[64 + p%64]
            P_stk = const.tile([128, TTW, H], bf16)
            nc.gpsimd.dma_start(out=P_stk[0:HN, :, :], in_=P_rep[HN:N, :, :])
            nc.gpsimd.dma_start(out=P_stk[HN:N, :, :], in_=P_rep[HN:N, :, :])
            # identity (bf16) for the P-route accumulate matmuls
            ident = const.tile([128, 128], bf16)
            make_identity(nc, ident)

            def stage_chunk(j0):
                # quadrant q (partition 32q) holds rows j0+2q, j0+2q+1
                chunk = stage.tile(
                    [128, RPQ * H], bf16, name=f"chunk_{j0}", tag="chunk"
                )
                nc.gpsimd.dma_start(
                    out=chunk[0:128:32, :],
                    in_=P_rep[j0 : j0 + JBLK, 0, :],
                )
                return chunk

            def next_routes(k):
                rs = [
                    ROUTES[(group_idx[0] + i) % len(ROUTES)] for i in range(k)
                ]
                group_idx[0] += k
                return rs

            def ident_mms(ps_t, rhs_P):
                # accumulate P into every slot of the tile on the PE
                for slot in range(TTW):
                    nc.tensor.matmul(
                        ps_t[:, slot * H : (slot + 1) * H],
                        ident,
                        rhs_P,
                        start=False,
                        stop=True,
                    )

            def finish_group(route, ps_t, out_sl, in0):
                ps_v = ps_t.rearrange("p (u h) -> p u h", u=TTW)
                if route[0] == "P":
                    # PE already added P; just copy PSUM -> bf16 out
                    if route[1] == "S":
                        nc.scalar.activation(
                            out_sl, ps_v, mybir.ActivationFunctionType.Copy
                        )
                    else:
                        nc.vector.tensor_copy(out_sl, ps_v)
                    return
                bc_t = bcast.tile([128, TTW * H], bf16, name="bc")
                if route[0] == "S":
                    nc.scalar.activation(
                        bc_t, ps_t, mybir.ActivationFunctionType.Copy
                    )
                else:
                    nc.vector.tensor_copy(bc_t, ps_t)
                eng = nc.gpsimd if route[1] == "G" else nc.vector
                eng.tensor_tensor(
                    out=out_sl,
                    in0=in0,
                    in1=bc_t.rearrange("p (u h) -> p u h", u=TTW),
                    op=mybir.AluOpType.add,
                )

            # ---- region 1: j-blocks with j < 64, full height, written to
            # the natural location AND (rows >= 64) to the mirror location
            def r1_block(jt):
                j0 = jt * JBLK
                chunk = stage_chunk(j0)
                out_tile = outp.tile([128, JBLK, H], bf16, name="out_r1")
                ps_a = psum.tile([128, TTW * H], f32, tag="ps", name="psa")
                ps_b = psum.tile([128, TTW * H], f32, tag="ps", name="psb")
                ps_tiles = [ps_a, ps_b]
                routes = next_routes(2)
                for s in range(RPQ):
                    for q in range(NQ):
                        ps_t = ps_tiles[q // 2]
                        slot = (q % 2) * RPQ + s
                        nc.tensor.matmul(
                            ps_t[:, slot * H : (slot + 1) * H],
                            ones_sb[q * 32 : q * 32 + 1, :],
                            chunk[q * 32 : q * 32 + 1, s * H : (s + 1) * H],
                            start=True,
                            stop=(routes[q // 2][0] != "P"),
                            tile_position=(q * 32, 0),
                        )
                for t, ps_t in enumerate(ps_tiles):
                    if routes[t][0] == "P":
                        ident_mms(ps_t, P_rep[:, 0, :])
                for t, ps_t in enumerate(ps_tiles):
                    finish_group(
                        routes[t],
                        ps_t,
                        out_tile[:, t * TTW : (t + 1) * TTW, :],
                        P_rep[:, :, :],
                    )
                nc.sync.dma_start(
                    out=out_ext[:, j0 : j0 + JBLK, :], in_=out_tile
                )
                # mirror: cell (i, j0+jj) -> (j0+jj, i) for i in [64, 128)
                base = out_ext[:, 0:JBLK, :]
                istep = base.ap[0][0]  # N*H
                jstep = base.ap[1][0]  # H
                mirror = bass.AP(
                    tensor=base.tensor,
                    offset=j0 * istep + HN * jstep,
                    ap=[[jstep, HN], [istep, JBLK], [1, H]],
                )
                nc.sync.dma_start(out=mirror, in_=out_tile[HN:N, :, :])

            # ---- region 2: (i >= 64, j >= 64) quadrant: half-height tiles,
            # two j-blocks packed per PSUM tile (col groups 0-1 vs 2-3)
            def r2_block(p4):
                jA = HN + 2 * p4 * JBLK
                jB = jA + JBLK
                chA = stage_chunk(jA)
                chB = stage_chunk(jB)
                out_tile = outp.tile([128, JBLK, H], bf16, name="out_r1")
                ps_a = psum.tile([128, TTW * H], f32, tag="ps", name="psa")
                ps_b = psum.tile([128, TTW * H], f32, tag="ps", name="psb")
                ps_tiles = [ps_a, ps_b]
                routes = next_routes(2)
                for s in range(RPQ):
                    for q in range(NQ):
                        ps_t = ps_tiles[q // 2]
                        slot = (q % 2) * RPQ + s
                        for half, ch in ((0, chA), (1, chB)):
                            nc.tensor.matmul(
                                ps_t[
                                    half * HN : (half + 1) * HN,
                                    slot * H : (slot + 1) * H,
                                ],
                                ones_sb[q * 32 : q * 32 + 1, 0:HN],
                                ch[q * 32 : q * 32 + 1, s * H : (s + 1) * H],
                                start=True,
                                stop=(routes[q // 2][0] != "P"),
                                tile_position=(q * 32, half * HN),
                            )
                for t, ps_t in enumerate(ps_tiles):
                    if routes[t][0] == "P":
                        ident_mms(ps_t, P_stk[:, 0, :])
                for t, ps_t in enumerate(ps_tiles):
                    finish_group(
                        routes[t],
                        ps_t,
                        out_tile[:, t * TTW : (t + 1) * TTW, :],
                        P_stk[:, :, :],
                    )
                # halves go to their own j-blocks (i rows 64..127)
                nc.sync.dma_start(
                    out=out_ext[HN:N, jA : jA + JBLK, :],
                    in_=out_tile[0:HN, :, :],
                )
                nc.sync.dma_start(
                    out=out_ext[HN:N, jB : jB + JBLK, :],
                    in_=out_tile[HN:N, :, :],
                )

            for jt in range(HN // JBLK):
                r1_block(jt)
            for k in range(4):
                r2_block(k)
    nc.compile()
    return nc


def _get_nc():
    if "nc" not in _BUILT:
        _BUILT["nc"] = _build_nc()
    return _BUILT["nc"]


def _make_in_maps(local_feats, W, b):
    import ml_dtypes

    bf = ml_dtypes.bfloat16
    local_feats = np.asarray(local_feats, dtype=np.float32)
    W = np.asarray(W, dtype=np.float32)
    b = np.asarray(b, dtype=np.float32)
    hb = np.ascontiguousarray((0.5 * b).reshape(1, H)).astype(bf)
    base = np.zeros((H, WXW), dtype=np.float32)
    base[:, N : N + H] = W.T
    base[0, N + H :] = 1.0
    in_maps = []
    for c in range(NCORES):
        wx = base.copy()
        wx[:, :N] = local_feats[c].T
        in_maps.append({"wx": wx.astype(bf), "halfb": hb})
    return in_maps


def _collect(res):
    return np.stack(
        [np.asarray(res.results[c]["out"]).astype(np.float32) for c in range(NCORES)],
        axis=0,
    )


def kernel(local_feats, W, b):
    from concourse.bass_utils import run_bass_kernel_spmd

    nc = _get_nc()
    in_maps = _make_in_maps(local_feats, W, b)
    res = run_bass_kernel_spmd(nc, in_maps, core_ids=list(range(NCORES)))
    return _collect(res)


def run_profiled(local_feats, W, b, **trace_kwargs):
    """Like kernel() but with neuron-profile tracing; returns (out, results)."""
    from concourse.bass_utils import run_bass_kernel_spmd

    nc = _get_nc()
    in_maps = _make_in_maps(local_feats, W, b)
    res = run_bass_kernel_spmd(
        nc, in_maps, core_ids=list(range(NCORES)), trace=True, **trace_kwargs
    )
    return _collect(res), res


# revision 73
# speedup vs baseline: 1.0383x; 1.0383x over previous
"""Trainium2 Bass kernel for nn_Attention_86199993631321.

Reference computation (B=8, N=128, H=512):
    pair[b,i,j,:] = x[b,i,:] + x[b,j,:]
    out = pair @ W.T + b                # [B, N, N, H]

Key algebraic simplification: the Linear is applied to a *sum*, so
    out[b,i,j,:] = P[b,i,:] + P[b,j,:]   where P = x @ W.T + 0.5*b
This turns 68.7 GFLOP of einsum into a 0.5 GFLOP matmul plus a broadcast-add
that only has to *write* the 268 MB output.

Sharding: pure data-parallel over batch B (core b handles batch b), no
collectives.  Per core:
  - P = x_b @ W.T + 0.5*b via TensorE (inputs pre-transposed and packed on
    host; the bias folds in as a K=1 matmul of a ones-row with 0.5*b).
  - P's rows are staged (bf16) into PE row-group base partitions {0,32,64,96};
    K=1 rank-1 matmuls ones ⊗ P[j,:] broadcast each row across partitions
    into PSUM, 4 row groups running concurrently in the array.
  - PSUM tiles are evicted as bf16 (ScalarE activation-copy or VectorE copy)
    and the i-term is added by bf16 2x-mode tensor_tensor on DVE or GpSimd.
  - out is symmetric (out[i,j]=out[j,i]), which saves 25% of the vector-engine
    work: j<64 tiles are computed at full height and ALSO written to their
    mirrored location (rows j, cols i>=64); the (i>=64, j>=64) quadrant is
    computed as half-height tiles packed two-per-PSUM-tile.
  - out lands in HBM as bf16 (half the write traffic); the host upcasts.
"""

import sys

if "/opt/trn_rl_repo" not in sys.path:
    sys.path.insert(0, "/opt/trn_rl_repo")

import numpy as np

B, N, H = 8, 128, 512
NCORES = 8
KC = H // 128  # contraction chunks for the P matmul
JBLK = 8       # j rows per output tile
TTW = 4        # j rows per PSUM tile ([128, TTW*H] = 4 banks)
NQ = 4         # PE row-group quadrants
RPQ = JBLK // NQ  # rows per quadrant in a chunk (2)
HN = N // 2    # 64
# packed input layout (per core, bf16): wx[h, 0:128] = x.T,
# wx[h, 128:640] = W.T, wx[0, 640:768] = 1.0 (ones row for the bias matmul)
WXW = N + H + 128
# Per-group eviction route (24 groups): SV = ACT copy + DVE TT, VV = DVE
# copy + DVE TT, PS/PV = the PE adds P itself via an identity matmul
# accumulation and the result needs only a copy (ACT / DVE).  The PE is
# mostly idle, so P-routes convert vector-engine work into array time.
# NOTE: GpSimd 2-input TTs steal DVE SBUF ports (concurrent DVE TTs degrade
# ~1.2us -> 3-5us), but a few SG routes still pay off by keeping the
# ACT-copy recycle cadence down.  Mix tuned empirically.
ROUTES = [
    "SV", "VV", "SG", "SV", "SV", "VV", "SG", "SV",
    "VV", "SV", "SG", "SV", "SV", "VV", "SG", "SV",
    "SV", "SV", "SG", "SV", "SV", "SG", "SV", "SV",
]

_BUILT = {}


def _build_nc():
    import concourse.bass as bass
    import concourse.bacc as bacc
    import concourse.tile as tile
    from concourse import mybir
    from concourse.masks import make_identity

    f32 = mybir.dt.float32
    bf16 = mybir.dt.bfloat16

    nc = bacc.Bacc()
    wx_ext = nc.declare_dram_parameter("wx", [H, WXW], bf16, isOutput=False)
    hb_ext = nc.declare_dram_parameter("halfb", [1, H], bf16, isOutput=False)
    out_ext = nc.declare_dram_parameter("out", [N, N, H], bf16, isOutput=True)

    group_idx = [0]

    with tile.TileContext(nc) as tc:
        with (
            tc.tile_pool(name="const", bufs=1) as const,
            tc.tile_pool(name="stage", bufs=6) as stage,
            tc.tile_pool(name="bcast", bufs=8) as bcast,
            tc.tile_pool(name="outp", bufs=6) as outp,
            tc.tile_pool(name="psum", bufs=2, space="PSUM") as psum,
        ):
            # ---- load packed inputs ----
            wx_sb = const.tile([128, KC, WXW], bf16)  # [h_local, (kc, m)]
            wx_v = wx_ext.rearrange("(c p) m -> p c m", p=128)
            # per-chunk DMAs on both HWDGE rings: the first proj matmul can
            # start as soon as its own K-chunk has landed
            for c in range(KC):
                eng = nc.sync if c % 2 == 0 else nc.scalar
                eng.dma_start(out=wx_sb[:, c, :], in_=wx_v[:, c, :])
            ones_sb = const.tile([128, 128], bf16)
            nc.vector.memset(ones_sb, 1.0)
            hb_sb = const.tile([1, H], bf16)
            nc.sync.dma_start(out=hb_sb, in_=hb_ext[:, :])

            # ---- P = x @ W.T + 0.5*b -> PSUM [128(i), 512(o)] ----
            ps_proj = psum.tile([128, TTW * H], f32, tag="ps")
            for c in range(KC):
                nc.tensor.matmul(
                    ps_proj[:, 0:H],
                    wx_sb[:, c, 0:N],
                    wx_sb[:, c, N : N + H],
                    start=(c == 0),
                    stop=False,
                )
            nc.tensor.matmul(
                ps_proj[:, 0:H],
                wx_sb[0:1, 0, N + H : N + H + 128],
                hb_sb,
                start=False,
                stop=True,
            )

            # P replicated 4x along the free dim (bf16): TT in0 + staging src
            P_rep = const.tile([128, TTW, H], bf16)
            for u in range(TTW):
                nc.scalar.activation(
                    P_rep[:, u, :],
                    ps_proj[:, 0:H],
                    mybir.ActivationFunctionType.Copy,
                )
            # stacked upper-half copy for the packed (i>=64) quadrant tiles:
            # P_stk[p] = P[64 + p%64]
            P_stk = const.tile([128, TTW, H], bf16)
            nc.gpsimd.dma_start(out=P_stk[0:HN, :, :], in_=P_rep[HN:N, :, :])
            nc.gpsimd.dma_start(out=P_stk[HN:N, :, :], in_=P_rep[HN:N, :, :])
            # identity (bf16) for the P-route accumulate matmuls
            ident = const.tile([128, 128], bf16)
            make_identity(nc, ident)

            def stage_chunk(j0):
                # quadrant q (partition 32q) holds rows j0+2q, j0+2q+1
                chunk = stage.tile(
                    [128, RPQ * H], bf16, name=f"chunk_{j0}", tag="chunk"
                )
                nc.gpsimd.dma_start(
                    out=chunk[0:128:32, :],
                    in_=P_rep[j0 : j0 + JBLK, 0, :],
                )
                return chunk

            def next_routes(k):
                rs = [
                    ROUTES[(group_idx[0] + i) % len(ROUTES)] for i in range(k)
                ]
                group_idx[0] += k
                return rs

            def ident_mms(ps_t, rhs_P):
                # accumulate P into every slot of the tile on the PE
                for slot in range(TTW):
                    nc.tensor.matmul(
                        ps_t[:, slot * H : (slot + 1) * H],
                        ident,
                        rhs_P,
                        start=False,
                        stop=True,
                    )

            def finish_group(route, ps_t, out_sl, in0):
                ps_v = ps_t.rearrange("p (u h) -> p u h", u=TTW)
                if route[0] == "P":
                    # PE already added P; just copy PSUM -> bf16 out
                    if route[1] == "S":
                        nc.scalar.activation(
                            out_sl, ps_v, mybir.ActivationFunctionType.Copy
                        )
                    else:
                        nc.vector.tensor_copy(out_sl, ps_v)
                    return
                bc_t = bcast.tile([128, TTW * H], bf16, name="bc")
                if route[0] == "S":
                    nc.scalar.activation(
                        bc_t, ps_t, mybir.ActivationFunctionType.Copy
                    )
                else:
                    nc.vector.tensor_copy(bc_t, ps_t)
                eng = nc.gpsimd if route[1] == "G" else nc.vector
                eng.tensor_tensor(
                    out=out_sl,
                    in0=in0,
                    in1=bc_t.rearrange("p (u h) -> p u h", u=TTW),
                    op=mybir.AluOpType.add,
                )

            # ---- region 1: j-blocks with j < 64, full height, written to
            # the natural location AND (rows >= 64) to the mirror location
            def r1_block(jt):
                j0 = jt * JBLK
                chunk = stage_chunk(j0)
                out_tile = outp.tile([128, JBLK, H], bf16, name="out_r1")
                ps_a = psum.tile([128, TTW * H], f32, tag="ps", name="psa")
                ps_b = psum.tile([128, TTW * H], f32, tag="ps", name="psb")
                ps_tiles = [ps_a, ps_b]
                routes = next_routes(2)
                for s in range(RPQ):
                    for q in range(NQ):
                        ps_t = ps_tiles[q // 2]
                        slot = (q % 2) * RPQ + s
                        nc.tensor.matmul(
                            ps_t[:, slot * H : (slot + 1) * H],
                            ones_sb[q * 32 : q * 32 + 1, :],
                            chunk[q * 32 : q * 32 + 1, s * H : (s + 1) * H],
                            start=True,
                            stop=(routes[q // 2][0] != "P"),
                            tile_position=(q * 32, 0),
                        )
                for t, ps_t in enumerate(ps_tiles):
                    if routes[t][0] == "P":
                        ident_mms(ps_t, P_rep[:, 0, :])
                for t, ps_t in enumerate(ps_tiles):
                    finish_group(
                        routes[t],
                        ps_t,
                        out_tile[:, t * TTW : (t + 1) * TTW, :],
                        P_rep[:, :, :],
                    )
                nc.sync.dma_start(
                    out=out_ext[:, j0 : j0 + JBLK, :], in_=out_tile
                )
                # mirror: cell (i, j0+jj) -> (j0+jj, i) for i in [64, 128)
                base = out_ext[:, 0:JBLK, :]
                istep = base.ap[0][0]  # N*H
                jstep = base.ap[1][0]  # H
                mirror = bass.AP(
                    tensor=base.tensor,
                    offset=j0 * istep + HN * jstep,
                    ap=[[jstep, HN], [istep, JBLK], [1, H]],
                )
                nc.sync.dma_start(out=mirror, in_=out_tile[HN:N, :, :])

            # ---- region 2: (i >= 64, j >= 64) quadrant: half-height tiles,
            # two j-blocks packed per PSUM tile (col groups 0-1 vs 2-3)
            def r2_block(p4):
                jA = HN + 2 * p4 * JBLK
                jB = jA + JBLK
                chA = stage_chunk(jA)
                chB = stage_chunk(jB)
                out_tile = outp.tile([128, JBLK, H], bf16, name="out_r1")
                ps_a = psum.tile([128, TTW * H], f32, tag="ps", name="psa")
                ps_b = psum.tile([128, TTW * H], f32, tag="ps", name="psb")
                ps_tiles = [ps_a, ps_b]
                routes = next_routes(2)
                for s in range(RPQ):
                    for q in range(NQ):
                        ps_t = ps_tiles[q // 2]
                        slot = (q % 2) * RPQ + s
                        for half, ch in ((0, chA), (1, chB)):
                            nc.tensor.matmul(
                                ps_t[
                                    half * HN : (half + 1) * HN,
                                    slot * H : (slot + 1) * H,
                                ],
                                ones_sb[q * 32 : q * 32 + 1, 0:HN],
                                ch[q * 32 : q * 32 + 1, s * H : (s + 1) * H],
                                start=True,
                                stop=(routes[q // 2][0] != "P"),
                                tile_position=(q * 32, half * HN),
                            )
                for t, ps_t in enumerate(ps_tiles):
                    if routes[t][0] == "P":
                        ident_mms(ps_t, P_stk[:, 0, :])
                for t, ps_t in enumerate(ps_tiles):
                    finish_group(
                        routes[t],
                        ps_t,
                        out_tile[:, t * TTW : (t + 1) * TTW, :],
                        P_stk[:, :, :],
                    )
                # halves go to their own j-blocks (i rows 64..127)
                nc.sync.dma_start(
                    out=out_ext[HN:N, jA : jA + JBLK, :],
                    in_=out_tile[0:HN, :, :],
                )
                nc.sync.dma_start(
                    out=out_ext[HN:N, jB : jB + JBLK, :],
                    in_=out_tile[HN:N, :, :],
                )

            for jt in range(HN // JBLK):
                r1_block(jt)
            for k in range(4):
                r2_block(k)
    nc.compile()
    return nc


def _get_nc():
    if "nc" not in _BUILT:
        _BUILT["nc"] = _build_nc()
    return _BUILT["nc"]


def _make_in_maps(local_feats, W, b):
    import ml_dtypes

    bf = ml_dtypes.bfloat16
    local_feats = np.asarray(local_feats, dtype=np.float32)
    W = np.asarray(W, dtype=np.float32)
    b = np.asarray(b, dtype=np.float32)
    hb = np.ascontiguousarray((0.5 * b).reshape(1, H)).astype(bf)
    base = np.zeros((H, WXW), dtype=np.float32)
    base[:, N : N + H] = W.T
    base[0, N + H :] = 1.0
    in_maps = []
    for c in range(NCORES):
        wx = base.copy()
        wx[:, :N] = local_feats[c].T
        in_maps.append({"wx": wx.astype(bf), "halfb": hb})
    return in_maps


def _collect(res):
    return np.stack(
        [np.asarray(res.results[c]["out"]).astype(np.float32) for c in range(NCORES)],
        axis=0,
    )


def kernel(local_feats, W, b):
    from concourse.bass_utils import run_bass_kernel_spmd

    nc = _get_nc()
    in_maps = _make_in_maps(local_feats, W, b)
    res = run_bass_kernel_spmd(nc, in_maps, core_ids=list(range(NCORES)))
    return _collect(res)


def run_profiled(local_feats, W, b, **trace_kwargs):
    """Like kernel() but with neuron-profile tracing; returns (out, results)."""
    from concourse.bass_utils import run_bass_kernel_spmd

    nc = _get_nc()
    in_maps = _make_in_maps(local_feats, W, b)
    res = run_bass_kernel_spmd(
        nc, in_maps, core_ids=list(range(NCORES)), trace=True, **trace_kwargs
    )
    return _collect(res), res
